# revision 22
# baseline (speedup 1.0000x reference)
"""MoE kernel v6: routed-token gather + mixed int8/bf16 weight streaming
with chunked, paced dequant interleaving.

Per-core (expert-parallel) pipeline:
  1. Exact fp32 router on all 256 tokens (gate col 0 = own expert after
     host-side roll) -> comb0[t] (combine weight, 0 if not routed here).
  2. Compaction positions via triangular-matmul prefix sum over the
     routed-token mask; one-hot P[t,j] tiles built with is_equal vs iota.
  3. Token gather as PE matmuls against TWO host-prescaled copies of x:
     xn1 = bf16(x * s1[h]) and xn3 = bf16(x * s3[h]) (the dequant scales
     of w1 / w3' ride the gather, so no per-group activation scaling ops
     exist at all).  Gathered [cap, H] transposed back to [h-part, cap]
     with PE transposes.  cap=128 token capacity.
  4. Expert MLP on gathered tokens in "flipped" orientation: gathered
     activations are the 128-col stationary, weight matrices stream as the
     512-wide moving operand (weight ingest at 1 col/cycle = PE floor,
     ~34 us measured).  PE inner order per weight group g: h1(g), hm
     transposes of g-1, h3(g), w2(g-1) - the ACT copy of the transposed
     hm lands during h3(g) so PE never waits on ACT.
  5. Weights stored row-normalized: wn1=w1/s1, wn3=(w3*s2)/s3, wn2=w2/s2
     (w2's per-row scale s2[i] folded into w3's columns).  Units (group,
     matrix) ship four ways: direct bf16 DMA (2x bytes, zero conv),
     gpsimd cast-DMA int8->bf16 (dequant rides the DMA engine), or int8 +
     DVE/ACT copy upconvert.  Engine-copied units are CHUNKED (1024/512
     cols) and paced through the DVE/ACT queues so the latency-critical
     silu/hm/hmt ops are never stuck behind a full-unit conversion.
  6. Combine + un-permute via Pw.T @ y matmul (Pw = comb-weighted one-hot);
     unrouted tokens come out exactly zero.  ReduceScatter over 8 cores.
"""

import sys

if "/opt/trn_rl_repo" not in sys.path:
    sys.path.insert(0, "/opt/trn_rl_repo")

import numpy as np

import concourse.bacc as bacc
import concourse.mybir as mybir
import concourse.tile as tile
from concourse.bass import ds as bass_ds, ts
from concourse.bass_utils import run_bass_kernel_spmd

T, H, I, E = 256, 1024, 4096, 8
N_CORES = 8
HK = H // 128  # 8 contraction chunks for w1/w3
TK = T // 128  # 2 token chunks (router, dense side)
CAP = 128  # routed-token capacity per expert (max actual load is 79)
GROUPS = 8  # w1/w3 streaming groups along I
IG = I // GROUPS  # 512
NS = 8  # w2 stages
SC = (I // 128) // NS  # 4 i-chunks per w2 stage

F32 = mybir.dt.float32
F32R = mybir.dt.float32r
BF16 = mybir.dt.bfloat16
I8 = mybir.dt.int8
AF = mybir.ActivationFunctionType
ALU = mybir.AluOpType
AX = mybir.AxisListType
BF16_NP = mybir.dt.np(BF16)
COMB_F32 = False  # partial sums + ReduceScatter in bf16 (fp32 out)

UNITS_ORDER = [(g, m) for g in range(GROUPS) for m in (1, 3, 2)]


# unit -> path map; units are (group, matrix) with matrix in {1, 3, 2}.
# direct: bf16 DMA (sync queue).  gpcast: int8 cast-DMA (gpsimd queue).
# vector/scalar: int8 DMA (sync) + chunked engine copy.
def default_sched():
    # scalar (ACT) gets NO conv units: its queue holds the latency-critical
    # silu/hmt/combine copies, and a conv chunk ahead of them gates PE.
    sched = {}
    for m in (1, 3, 2):
        sched[(0, m)] = "direct"
        sched[(1, m)] = "direct"
        sched[(2, m)] = "direct"
    for g in range(3, GROUPS):
        sched[(g, 1)] = "vector"
        sched[(g, 3)] = "gpcast"
        sched[(g, 2)] = "gpcast" if g <= 4 else "vector"
    return sched


# conv chunk sizes (columns of the [128, 4096] unit) and per-piece pacing
VCHUNK, SCHUNK = 1024, 512
PACE_V, PACE_S = 5, 4


def build_nc(
    iters: int = 1,
    n_cores: int = N_CORES,
    with_collective: bool = True,
    combine: str = "rs",
    comb_f32: bool = COMB_F32,
    sched: dict | None = None,
    dma_ahead: int = 3,
    mlp_only: bool = False,
):
    nc = bacc.Bacc("TRN2", target_bir_lowering=False, debug=False, num_devices=n_cores)
    if sched is None:
        sched = default_sched()
    direct_units = [u for u in UNITS_ORDER if sched[u] == "direct"]
    dcol = {u: i * 4096 for i, u in enumerate(direct_units)}

    xT32 = nc.dram_tensor("xT32", [H, T], F32, kind="ExternalInput")
    xn1 = nc.dram_tensor("xn1", [T, H], BF16, kind="ExternalInput")
    xn3 = nc.dram_tensor("xn3", [T, H], BF16, kind="ExternalInput")
    gate = nc.dram_tensor("gate", [H, E], F32, kind="ExternalInput")
    # merged per-group weight wall (host pre-shuffled): per partition row,
    # cols [0:4096)=w1 [HK,IG], [4096:8192)=w3', [8192:12288)=w2 [SC,H]
    PER = HK * IG + HK * IG + SC * H
    wall = nc.dram_tensor("wall", [GROUPS * 128, PER], I8, kind="ExternalInput")
    if direct_units:
        wall16 = nc.dram_tensor(
            "wall16", [128, len(direct_units) * 4096], BF16, kind="ExternalInput"
        )
    trid = nc.dram_tensor("tri", [128, 128], F32, kind="ExternalInput")
    onesd = nc.dram_tensor("ones", [128, 128], F32, kind="ExternalInput")
    idbd = nc.dram_tensor("idb", [128, 128], BF16, kind="ExternalInput")
    idfd = nc.dram_tensor("idf", [128, 128], F32, kind="ExternalInput")

    TS = T // n_cores
    OUT_DT = F32
    if combine == "rs" and with_collective:
        out = nc.dram_tensor("out", [TS, H], OUT_DT, kind="ExternalOutput")
    else:
        out = nc.dram_tensor("out", [T, H], OUT_DT, kind="ExternalOutput")

    xT32_v = xT32.ap().rearrange("(ho hi) t -> hi ho t", hi=128)
    xn1_v = xn1.ap().rearrange("(tk ti) h -> ti tk h", ti=128)
    xn3_v = xn3.ap().rearrange("(tk ti) h -> ti tk h", ti=128)
    gate_v = gate.ap().rearrange("(ho hi) e -> hi ho e", hi=128)

    with tile.TileContext(nc) as tc:
        with (
            tc.tile_pool(name="consts", bufs=1) as consts,
            tc.tile_pool(name="zpool", bufs=2) as zpool,
            tc.tile_pool(name="wq1", bufs=3) as wq1,
            tc.tile_pool(name="wq3", bufs=3) as wq3,
            tc.tile_pool(name="wq2", bufs=3) as wq2,
            tc.tile_pool(name="wb1", bufs=3) as wb1,
            tc.tile_pool(name="wb3", bufs=3) as wb3,
            tc.tile_pool(name="wb2", bufs=3) as wb2,
            tc.tile_pool(name="hpool", bufs=6) as hpool,
            tc.tile_pool(name="small", bufs=2) as small,
            tc.tile_pool(name="gath", bufs=2) as gath,
            tc.tile_pool(name="outsb", bufs=2) as outsb,
            tc.tile_pool(name="ps_a", bufs=2, space="PSUM") as ps_a,
            tc.tile_pool(name="ps_b", bufs=2, space="PSUM") as ps_b,
            tc.tile_pool(name="ps_big", bufs=1, space="PSUM") as ps_big,
            tc.tile_pool(name="ps_tr", bufs=2, space="PSUM") as ps_tr,
            tc.tile_pool(name="dram", bufs=1, space="DRAM") as dram,
        ):
            CBDT = F32 if comb_f32 else BF16
            partial = dram.tile([T, H], CBDT)
            if combine == "rs":
                reduced = dram.tile([TS, H], CBDT)
            else:
                reduced = dram.tile([T, H], CBDT)

            # ---- constants (loaded once, scalar HWDGE queue) ----
            tri_sb = consts.tile([128, 128], F32, tag="tri")
            ones_sb = consts.tile([128, 128], F32, tag="ones")
            idb_sb = consts.tile([128, 128], BF16, tag="idb")
            idf_sb = consts.tile([128, 128], F32, tag="idf")
            nc.scalar.dma_start(tri_sb[:], trid.ap())
            nc.scalar.dma_start(ones_sb[:], onesd.ap())
            nc.scalar.dma_start(idb_sb[:], idbd.ap())
            nc.scalar.dma_start(idf_sb[:], idfd.ap())
            iota_sb = consts.tile([128, CAP], F32, tag="iota")
            nc.gpsimd.iota(
                iota_sb[:],
                pattern=[[1, CAP]],
                base=0,
                channel_multiplier=0,
                allow_small_or_imprecise_dtypes=True,
            )

            W13 = HK * IG
            MSEC = {1: (0, W13), 3: (W13, 2 * W13), 2: (2 * W13, PER)}
            MPOOLS = {1: (wq1, wb1), 3: (wq3, wb3), 2: (wq2, wb2)}

            def body(_iv=None):
                # ---- activation loads (sync queue): gate first (tiny),
                # z32 in two halves so the router's first contraction chunks
                # start after half the transfer; group 0's weights slot in
                # before the gather sources (needed ~5us later).
                z32a = zpool.tile([128, HK // 2, T], F32, tag="z32a")
                z32b = zpool.tile([128, HK // 2, T], F32, tag="z32b")
                xg1 = zpool.tile([128, TK, H], BF16, tag="xn1")
                xg3 = zpool.tile([128, TK, H], BF16, tag="xn3")
                g_sb = zpool.tile([128, HK, E], F32, tag="g")
                if not mlp_only:
                    nc.sync.dma_start(g_sb[:], gate_v)
                    nc.sync.dma_start(z32a[:], xT32_v[:, 0 : HK // 2, :])
                    nc.sync.dma_start(z32b[:], xT32_v[:, HK // 2 : HK, :])

                def z32v(hk):
                    return (
                        z32a[:, hk, :] if hk < HK // 2 else z32b[:, hk - HK // 2, :]
                    )

                w1b, w3b, w2b = {}, {}, {}
                wviews = {1: w1b, 3: w3b, 2: w2b}
                # per-engine chunked conversion queues (consumption order)
                chq = {"vector": [], "scalar": []}
                def dma_w(g):
                    for m in (1, 3, 2):
                        lo, hi = MSEC[m]
                        qpool, bpool = MPOOLS[m]
                        bt = bpool.tile([128, 4096], BF16, tag="b")
                        kind = sched[(g, m)]
                        if kind == "direct":
                            nc.sync.dma_start(
                                bt[:],
                                wall16.ap()[:, bass_ds(dcol[(g, m)], 4096)],
                            )
                        elif kind == "gpcast":
                            nc.gpsimd.dma_start(
                                bt[:], wall.ap()[ts(g, 128), lo:hi]
                            )
                        else:
                            qt = qpool.tile([128, 4096], I8, tag="q")
                            nc.sync.dma_start(
                                qt[:], wall.ap()[ts(g, 128), lo:hi]
                            )
                            csz = VCHUNK if kind == "vector" else SCHUNK
                            for c0 in range(0, 4096, csz):
                                chq[kind].append((g, bt, qt, c0, csz))
                        if m == 2:
                            wviews[m][g] = bt[:].rearrange(
                                "p (ko h) -> p ko h", ko=SC
                            )
                        else:
                            wviews[m][g] = bt[:].rearrange(
                                "p (ho i) -> p ho i", ho=HK
                            )

                def emit_chunk(eng, ent):
                    _, bt, qt, c0, csz = ent
                    sl = bass_ds(c0, csz)
                    if eng == "scalar":
                        nc.scalar.copy(bt[:, sl], qt[:, sl])
                    else:
                        nc.vector.tensor_copy(bt[:, sl], qt[:, sl])

                def pace(eng, n):
                    q = chq[eng]
                    for _ in range(min(n, len(q))):
                        emit_chunk(eng, q.pop(0))

                def drain_upto(gmax):
                    for eng in ("vector", "scalar"):
                        q = chq[eng]
                        while q and q[0][0] <= gmax:
                            emit_chunk(eng, q.pop(0))

                dma_w(0)
                if not mlp_only:
                    nc.sync.dma_start(xg1[:], xn1_v)
                    nc.sync.dma_start(xg3[:], xn3_v)
                for g in range(1, min(dma_ahead, GROUPS)):
                    dma_w(g)

                def run_mlp(zg1, zg3, pwt):
                    # ---- expert MLP on gathered tokens (flipped orientation)
                    out_ps = ps_big.tile([128, H], F32, tag="big")
                    NCH = IG // 128
                    hm_tiles = {}
                    hmt_tiles = {}

                    def w2_tr(piece):
                        isl, c0, nch = piece
                        hmt_ps = ps_tr.tile([128, NCH, CAP], BF16, tag="tr")
                        for c in range(nch):
                            nc.tensor.transpose(
                                hmt_ps[:, c, :],
                                hm_tiles[piece][:, ts(c, 128)],
                                idb_sb[:],
                            )
                        hmt = hpool.tile([128, NCH, CAP], BF16, tag="hmt")
                        nc.scalar.copy(hmt[:, 0:nch, :], hmt_ps[:, 0:nch, :])
                        hmt_tiles[piece] = hmt

                    def w2_mm(piece):
                        isl, c0, nch = piece
                        hmt = hmt_tiles[piece]
                        for c in range(nch):
                            m = isl * NCH + c0 + c  # global i-chunk 0..31
                            s, off = divmod(m, SC)
                            for n in range(2):
                                nc.tensor.matmul(
                                    out_ps[:, ts(n, 512)],
                                    hmt[:, c, :],
                                    w2b[s][:, off, ts(n, 512)],
                                    start=(m == 0),
                                    stop=(m == I // 128 - 1),
                                )

                    # last weight group split in half so the final dependency
                    # chain (dma -> h1/h3 -> silu -> hm -> transpose -> w2)
                    # runs on half-size tiles
                    pieces = [(isl, 0, NCH) for isl in range(GROUPS - 2)]
                    for gl in (GROUPS - 2, GROUPS - 1):
                        pieces += [
                            (gl, 0, NCH // 2),
                            (gl, NCH // 2, NCH - NCH // 2),
                        ]

                    for pi, piece in enumerate(pieces):
                        isl, c0, nch = piece
                        w = nch * 128
                        if c0 == 0 and isl + dma_ahead < GROUPS:
                            dma_w(isl + dma_ahead)
                        drain_upto(isl)
                        h1 = ps_a.tile([128, IG], F32, tag="a")
                        h3 = ps_b.tile([128, IG], F32, tag="b")
                        for hk in range(HK):
                            nc.tensor.matmul(
                                h1[:, 0:w],
                                zg1[:, hk, :],
                                w1b[isl][:, hk, bass_ds(c0 * 128, w)],
                                start=(hk == 0),
                                stop=(hk == HK - 1),
                            )
                        # hm transposes of the previous piece run between the
                        # h1 and h3 chains; their ACT copy lands during h3.
                        if pi >= 1:
                            w2_tr(pieces[pi - 1])
                        for hk in range(HK):
                            nc.tensor.matmul(
                                h3[:, 0:w],
                                zg3[:, hk, :],
                                w3b[isl][:, hk, bass_ds(c0 * 128, w)],
                                start=(hk == 0),
                                stop=(hk == HK - 1),
                            )
                        if pi >= 1:
                            w2_mm(pieces[pi - 1])
                        h1s = hpool.tile([128, IG], F32, tag="h1s")
                        nc.scalar.activation(h1s[:, 0:w], h1[:, 0:w], AF.Silu)
                        hm = hpool.tile([128, IG], BF16, tag="hm")
                        nc.vector.tensor_mul(hm[:, 0:w], h1s[:, 0:w], h3[:, 0:w])
                        hm_tiles[piece] = hm
                        pace("vector", PACE_V)
                        pace("scalar", PACE_S)
                    w2_tr(pieces[-1])
                    w2_mm(pieces[-1])

                    # ---- combine + un-permute: partial = PwT @ y
                    y_sb = gath.tile([128, H], F32R, tag="y")
                    o_sbs = [
                        outsb.tile([128, H], CBDT, tag=f"o{t}", name=f"o_sb{t}")
                        for t in range(TK)
                    ]
                    for n in range(2):
                        nc.vector.tensor_copy(
                            y_sb[:, ts(n, 512)], out_ps[:, ts(n, 512)]
                        )
                        for t in range(TK):
                            up = ps_a.tile([128, 512], F32, tag="a")
                            nc.tensor.matmul(
                                up[:],
                                pwt[t][:],
                                y_sb[:, ts(n, 512)],
                                start=True,
                                stop=True,
                            )
                            if t == 0:
                                nc.scalar.copy(o_sbs[t][:, ts(n, 512)], up[:])
                            else:
                                nc.vector.tensor_copy(
                                    o_sbs[t][:, ts(n, 512)], up[:]
                                )
                    for t in range(TK):
                        nc.sync.dma_start(partial[ts(t, 128), :], o_sbs[t][:])

                if mlp_only:
                    zg1 = gath.tile([128, HK, CAP], BF16, tag="zg1")
                    zg3 = gath.tile([128, HK, CAP], BF16, tag="zg3")
                    nc.sync.dma_start(
                        zg1[:].rearrange("p a b -> p (a b)"), xn1_v[:, 0, :]
                    )
                    nc.sync.dma_start(
                        zg3[:].rearrange("p a b -> p (a b)"), xn3_v[:, 0, :]
                    )
                    pwt = []
                    for t in range(TK):
                        pw_sb = gath.tile([128, 128], F32R, tag=f"pwt{t}")
                        nc.vector.tensor_copy(pw_sb[:], tri_sb[:])
                        pwt.append(pw_sb)
                    run_mlp(zg1, zg3, pwt)
                    return

                # ---- router (exact fp32), comb0[t] per token chunk
                comb0 = []
                for t in range(TK):
                    ps_r = ps_a.tile([128, E], F32, tag="a")
                    for hk in range(HK):
                        nc.tensor.matmul(
                            ps_r[:],
                            z32v(hk)[:, ts(t, 128)],
                            g_sb[:, hk, :],
                            start=(hk == 0),
                            stop=(hk == HK - 1),
                        )
                    neg_mx = small.tile([128, 1], F32, tag="neg_mx")
                    nc.vector.tensor_reduce(
                        neg_mx[:], ps_r[:], AX.X, ALU.max, negate=True
                    )
                    ex = small.tile([128, E], F32, tag="ex")
                    nc.scalar.activation(ex[:], ps_r[:], AF.Exp, bias=neg_mx[:])
                    ssum = small.tile([128, 1], F32, tag="ssum")
                    nc.vector.tensor_reduce(ssum[:], ex[:], AX.X, ALU.add)
                    srec = small.tile([128, 1], F32, tag="srec")
                    nc.vector.reciprocal(srec[:], ssum[:])
                    p = small.tile([128, E], F32, tag="p")
                    nc.vector.tensor_scalar_mul(p[:], ex[:], srec[:])
                    m1 = small.tile([128, 1], F32, tag="m1")
                    nc.vector.tensor_reduce(m1[:], p[:], AX.X, ALU.max)
                    pm = small.tile([128, E], F32, tag="pm")
                    nc.vector.tensor_single_scalar(pm[:], p[:], m1[:], ALU.is_equal)
                    p2 = small.tile([128, E], F32, tag="p2")
                    nc.vector.scalar_tensor_tensor(
                        p2[:], pm[:], -2.0, p[:], ALU.mult, ALU.add
                    )
                    m2 = small.tile([128, 1], F32, tag="m2")
                    nc.vector.tensor_reduce(m2[:], p2[:], AX.X, ALU.max)
                    denom = small.tile([128, 1], F32, tag="denom")
                    nc.vector.tensor_add(denom[:], m1[:], m2[:])
                    drec = small.tile([128, 1], F32, tag="drec")
                    nc.vector.reciprocal(drec[:], denom[:])
                    sel = small.tile([128, 1], F32, tag="sel")
                    nc.vector.tensor_single_scalar(sel[:], p[:, 0:1], m2[:], ALU.is_ge)
                    wn = small.tile([128, 1], F32, tag="wn")
                    nc.vector.tensor_scalar_mul(wn[:], p[:, 0:1], drec[:])
                    cb = small.tile([128, 1], F32, tag="cb")
                    nc.vector.tensor_mul(cb[:], wn[:], sel[:])
                    comb0.append(cb)

                # ---- compaction positions: pos = prefix-sum of mask
                masks = []
                for t in range(TK):
                    mk = small.tile([128, 1], F32, tag=f"mk{t}")
                    nc.vector.tensor_single_scalar(mk[:], comb0[t][:], 0.0, ALU.is_gt)
                    masks.append(mk)
                posm = []
                for t in range(TK):
                    pp = ps_a.tile([128, 1], F32, tag="a")
                    if t == 0:
                        nc.tensor.matmul(
                            pp[:], tri_sb[:], masks[0][:], start=True, stop=True
                        )
                    else:
                        nc.tensor.matmul(
                            pp[:], ones_sb[:], masks[0][:], start=True, stop=False
                        )
                        nc.tensor.matmul(
                            pp[:], tri_sb[:], masks[1][:], start=False, stop=True
                        )
                    pm_t = small.tile([128, 1], F32, tag=f"pm{t}")
                    nc.vector.tensor_mul(pm_t[:], pp[:], masks[t][:])
                    pmm = small.tile([128, 1], F32, tag=f"pmm{t}")
                    nc.vector.tensor_scalar_add(pmm[:], pm_t[:], -1.0)
                    posm.append(pmm)

                # ---- one-hot P (bf16) and comb-weighted Pw (fp32)
                P_bf, Pw = [], []
                for t in range(TK):
                    pb = gath.tile([128, CAP], BF16, tag=f"pb{t}")
                    nc.vector.tensor_tensor(
                        pb[:],
                        posm[t][:, 0:1].to_broadcast([128, CAP]),
                        iota_sb[:],
                        ALU.is_equal,
                    )
                    P_bf.append(pb)
                    pw = gath.tile([128, CAP], F32, tag=f"pw{t}")
                    nc.vector.tensor_scalar_mul(pw[:], pb[:], comb0[t][:])
                    Pw.append(pw)

                # ---- gathers: zgT = P.T @ xn{1,3}, transposed to [h, cap]
                def gather(xg, tagz):
                    zgt_ps = ps_big.tile([128, H], F32, tag="big")
                    for t in range(TK):
                        for n in range(2):
                            nc.tensor.matmul(
                                zgt_ps[:, ts(n, 512)],
                                P_bf[t][:],
                                xg[:, t, ts(n, 512)],
                                start=(t == 0),
                                stop=(t == TK - 1),
                            )
                    zgt_sb = gath.tile([128, H], BF16, tag=f"zgt{tagz}")
                    nc.vector.tensor_copy(zgt_sb[:], zgt_ps[:])
                    zg_ps = ps_big.tile([128, HK, CAP], BF16, tag="big")
                    for k in range(HK):
                        nc.tensor.transpose(
                            zg_ps[:, k, :], zgt_sb[:, ts(k, 128)], idb_sb[:]
                        )
                    zg = gath.tile([128, HK, CAP], BF16, tag=f"zg{tagz}")
                    nc.vector.tensor_copy(zg[:], zg_ps[:])
                    return zg

                zg1 = gather(xg1, "1")
                zg3 = gather(xg3, "3")

                # ---- transpose the comb-weighted one-hot now (off the tail)
                pwt = []
                for t in range(TK):
                    pwt_ps = ps_tr.tile([128, 128], F32, tag="tr")
                    nc.tensor.transpose(pwt_ps[:], Pw[t][:], idf_sb[:])
                    pw_sb = gath.tile([128, 128], F32R, tag=f"pwt{t}")
                    nc.vector.tensor_copy(pw_sb[:], pwt_ps[:])
                    pwt.append(pw_sb)

                run_mlp(zg1, zg3, pwt)

            if iters == 1:
                body()
            else:
                with tc.For_i(0, iters, 1, hint_engines=(mybir.EngineType.PE,)) as iv:
                    body(iv)

            if with_collective:
                nc.gpsimd.collective_compute(
                    "ReduceScatter" if combine == "rs" else "AllReduce",
                    ALU.add,
                    replica_groups=[list(range(n_cores))],
                    ins=[partial[:].opt()],
                    outs=[reduced[:].opt()],
                )
                if CBDT == OUT_DT:
                    nc.sync.dma_start(out[:], reduced[:])
                else:
                    rs_sb = outsb.tile([TS, H], CBDT, tag="rs")
                    nc.sync.dma_start(rs_sb[:], reduced[:])
                    rs32 = outsb.tile([TS, H], OUT_DT, tag="rs32")
                    nc.vector.tensor_copy(rs32[:], rs_sb[:])
                    nc.sync.dma_start(out[:], rs32[:])
            else:
                nc.gpsimd.dma_start(out[:], partial[:])

    nc.compile()
    return nc


def quantize_rows(w):
    """Per-row normalization: returns (normalized fp32, scales)."""
    s = np.abs(w).max(axis=1) / 127.0
    s = np.maximum(s, 1e-12)
    return w / s[:, None], s.astype(np.float32)


def make_in_maps(hidden_states, gate_w, w1s, w2s, w3s, n_cores=N_CORES, sched=None):
    if sched is None:
        sched = default_sched()
    direct_units = [u for u in UNITS_ORDER if sched[u] == "direct"]
    x32 = np.asarray(hidden_states, np.float32)
    xT = np.ascontiguousarray(x32.T)
    gate_w = np.asarray(gate_w, np.float32)
    w1s = np.asarray(w1s, np.float32)
    w2s = np.asarray(w2s, np.float32)
    w3s = np.asarray(w3s, np.float32)
    tri = np.triu(np.ones((128, 128), np.float32))
    ones = np.ones((128, 128), np.float32)
    idb = np.eye(128, dtype=np.float32).astype(BF16_NP)
    idf = np.eye(128, dtype=np.float32)

    in_maps = []
    for c in range(n_cores):
        w1c, w2c, w3c = w1s[c], w2s[c], w3s[c]
        wn2, s2 = quantize_rows(w2c)  # [I, H] rows over h -> s2[i]
        # fold s2[i] into w3's columns BEFORE normalizing w3: the streamed
        # w2 then needs no runtime dequant scale at all.
        w3p = w3c * s2[None, :].astype(np.float32)
        wn1, s1 = quantize_rows(w1c)  # [H, I] rows over i -> s1[h]
        wn3, s3 = quantize_rows(w3p)

        def to_groups13(wn):
            return np.ascontiguousarray(
                wn.reshape(HK, 128, GROUPS, IG).transpose(2, 1, 0, 3)
            ).reshape(GROUPS, 128, HK * IG)

        def to_groups2(wn):
            return np.ascontiguousarray(
                wn.reshape(NS, SC, 128, H).transpose(0, 2, 1, 3)
            ).reshape(NS, 128, SC * H)

        g1, g3, g2 = to_groups13(wn1), to_groups13(wn3), to_groups2(wn2)
        gm = {1: g1, 3: g3, 2: g2}
        wallc = np.ascontiguousarray(
            np.concatenate(
                [
                    np.clip(np.round(g1), -127, 127),
                    np.clip(np.round(g3), -127, 127),
                    np.clip(np.round(g2), -127, 127),
                ],
                axis=2,
            )
        ).astype(np.int8).reshape(GROUPS * 128, -1)
        m = {
            "xT32": xT,
            "xn1": (x32 * s1[None, :]).astype(BF16_NP),
            "xn3": (x32 * s3[None, :]).astype(BF16_NP),
            "gate": np.ascontiguousarray(np.roll(gate_w, -c, axis=1)),
            "wall": wallc,
            "tri": tri,
            "ones": ones,
            "idb": idb,
            "idf": idf,
        }
        if direct_units:
            # direct units: bf16 of the normalized fp32 (skips int8 rounding)
            m["wall16"] = np.ascontiguousarray(
                np.concatenate([gm[mt][g] for (g, mt) in direct_units], axis=1)
            ).astype(BF16_NP)
        in_maps.append(m)
    return in_maps


_CACHE = {}


def _built(key):
    if key not in _CACHE:
        _CACHE[key] = build_nc(*key)
    return _CACHE[key]


def kernel(hidden_states, gate_w, w1s, w2s, w3s):
    in_maps = make_in_maps(hidden_states, gate_w, w1s, w2s, w3s)
    nc = _built((1, N_CORES, True))
    res = run_bass_kernel_spmd(nc, in_maps, core_ids=list(range(N_CORES)))
    return np.concatenate(
        [np.asarray(res.results[c]["out"]) for c in range(N_CORES)], axis=0
    ).astype(np.float32, copy=False)
